# revision 58
# baseline (speedup 1.0000x reference)
"""AdaptiveBlockSparseAttnTrain Trainium2 kernel (8 NeuronCores, head-parallel).

Per core (= one head), fused single pass over query-block groups with a
software-pipelined PE schedule:
  - Gilbert rearrange/unrearrange, padding, transposes, final division done
    host-side (cheap numpy); device computes the attention pipeline.
  - ST_ij = K_j @ Q_group^T on TensorE in fp16, two key blocks per PSUM
    bank pair; E = exp(ST * scale) on ScalarE in 1024-wide batches.
  - W[j, q] (pooled mass per key block) via accumulating basis matmuls; the
    basis carries a 32nd all-ones column so the full softmax denominator
    den[q] lands in W row 31 for free.
  - Energy mask, rank-based (equals reference argsort/cumsum/clip for
    tie-free inputs):  keep = (cum_incl < 0.95*tot & rank < 21) | rank < 1.
  - Mask multiply in place on E, one DVE pass per query block with
    pair-packed APs (a pair-duplicated mask tile gives every operand a
    unit-stride 2-element innermost dim -> DVE 2x mode). E stays j-major
    because strided moving/activation APs are 2-3x slower on real hardware
    than the cost model predicts.
  - PV transposed: O^T_group[d, q] += V_j^T @ E_masked_j, interleaved into
    the NEXT group's ST phase so TensorE never idles while the mask chain
    (tiny PE transposes + DVE compares) runs.
  - Masked denominator via fused DVE accumulate; final division by it and
    the [d, q] -> [q, d] transpose happen on the host in f32 (output f16).
"""

import sys
import types

sys.path.insert(0, "/opt/trn_rl_repo")

import numpy as np

# The NTFF profile path (fired when BASS_TRACE is set in the environment)
# imports antenv.axon_hooks, which this image does not ship. Register a stub
# so run_bass_kernel_spmd degrades gracefully (skips tracing) instead of
# crashing, without shadowing a real module if one exists.
try:
    import antenv.axon_hooks  # noqa: F401
except ImportError:
    _m = types.ModuleType("antenv.axon_hooks")
    _hook = {}
    _m.set_axon_ntff_profile_hook = lambda h: _hook.__setitem__("h", h)
    _m.get_axon_ntff_profile_hook = lambda: _hook.get("h")
    sys.modules["antenv.axon_hooks"] = _m

import concourse.bass as bass
import concourse.bacc as bacc
import concourse.tile as tile
from concourse import mybir
from concourse.bass_utils import run_bass_kernel_spmd

TEXT = 224
VID = 3696
SEQ = 3920
BLOCK = 128
NB = 31
SP = 3968
D = 128
NCORES = 8
NVLAST = SEQ - 30 * 128        # 80 valid tokens in the last block
SCALE = 1.0 / np.sqrt(128.0)

F32 = mybir.dt.float32
F16 = mybir.dt.float16

GROUPS = [(0, 4), (4, 4), (8, 4), (12, 4), (16, 4), (20, 4), (24, 4), (28, 3)]
PAIRS = [(j, j + 1) for j in range(0, 30, 2)] + [(30,)]


def _ap(base, dims):
    """Rebuild a tile-slice AP with custom free dims (keeps partition dim +
    offset). dims = [[step, count], ...] in elements, outermost first."""
    return bass.AP(
        tensor=base.tensor,
        offset=base.offset,
        ap=[list(base.ap[0])] + [list(d) for d in dims],
    )


def build_graph():
    nc = bacc.Bacc("TRN2", target_bir_lowering=False, debug=False,
                   num_devices=NCORES)
    qT_d = nc.dram_tensor("qT", [128, SP], F16, kind="ExternalInput").ap()
    kT_d = nc.dram_tensor("kT", [128, SP], F16, kind="ExternalInput").ap()
    vv_d = nc.dram_tensor("vv", [128, NB * 128], F16, kind="ExternalInput").ap()
    bas_d = nc.dram_tensor("bas", [128, NB * 32], F16, kind="ExternalInput").ap()
    i32_d = nc.dram_tensor("i32", [32, 32], F32, kind="ExternalInput").ap()
    i128_d = nc.dram_tensor("i128", [128, 128], F32, kind="ExternalInput").ap()
    outT_d = nc.dram_tensor("outT", [128, SP], F16, kind="ExternalOutput").ap()
    den_d = nc.dram_tensor("den", [128, NB], F32, kind="ExternalOutput").ap()

    with tile.TileContext(nc) as tc:
        with (
            tc.tile_pool(name="stp", bufs=2, space="PSUM") as stpool,
            tc.tile_pool(name="wps", bufs=1, space="PSUM") as wps,
            tc.tile_pool(name="pvps", bufs=1, space="PSUM") as pvps,
            tc.tile_pool(name="mini", bufs=2, space="PSUM") as minips,
            tc.tile_pool(name="singles", bufs=1) as singles,
            tc.tile_pool(name="eg", bufs=3) as egp,
            tc.tile_pool(name="gw", bufs=2) as gwp,
            tc.tile_pool(name="small", bufs=2) as small,
            tc.tile_pool(name="outs", bufs=2) as outsp,
        ):
            # ---- PE warmup: keep TensorE busy (p-state ramp) during DMAs
            warm = singles.tile([128, 512], F16)
            nc.vector.memset(warm[:, :], 0.0)
            warmP = stpool.tile([128, 1024], F32, tag="st", name="warmP")
            for _ in range(12):
                nc.tensor.matmul(warmP[:, 0:512], warm[:, 0:128], warm[:, :],
                                 start=True, stop=True)

            # ---- resident inputs (chunked: unblock group 0 ASAP) ----
            sq = singles.tile([128, SP], F16)
            sk = singles.tile([128, SP], F16)
            sv = singles.tile([128, NB, 128], F16)
            sbas = singles.tile([128, NB, 32], F16)
            i32 = singles.tile([32, 32], F32)
            i128 = singles.tile([128, 128], F32)
            nc.sync.dma_start(sbas[:, :, :],
                              bas_d.rearrange("p (j m) -> p j m", j=NB))
            nc.sync.dma_start(sk[:, 0:512], kT_d[:, 0:512])
            nc.sync.dma_start(sq[:, 0:512], qT_d[:, 0:512])
            nc.sync.dma_start(i32[:, :], i32_d)
            nc.sync.dma_start(sk[:, 512:1536], kT_d[:, 512:1536])
            nc.sync.dma_start(sk[:, 1536:2560], kT_d[:, 1536:2560])
            nc.sync.dma_start(sk[:, 2560:SP], kT_d[:, 2560:SP])
            nc.sync.dma_start(sq[:, 512:1536], qT_d[:, 512:1536])
            nc.sync.dma_start(sq[:, 1536:2560], qT_d[:, 1536:2560])
            nc.sync.dma_start(sq[:, 2560:SP], qT_d[:, 2560:SP])
            nc.sync.dma_start(sv[:, :, :],
                              vv_d.rearrange("p (j w) -> p j w", j=NB))
            nc.sync.dma_start(i128[:, :], i128_d)

            # ---- constants ----
            ones_col128 = singles.tile([128, 1], F32)
            nc.vector.memset(ones_col128[:, :], 1.0)
            onesq128 = singles.tile([128, 128], F16)
            nc.vector.memset(onesq128[:, :], 1.0)
            mrowB = singles.tile([128, 128], F16)
            nc.vector.memset(mrowB[:, :], 0.0)
            ones_row31 = singles.tile([1, 31], F32)
            nc.vector.memset(ones_row31[:, :], 1.0)
            onesb31 = singles.tile([128, 31], F32)
            nc.vector.memset(onesb31[:, :], 1.0)
            den_sb = singles.tile([128, NB], F32)
            nc.vector.memset(den_sb[:, :], 1.0)

            egs = {}
            wsbs = {}
            mbcs = {}
            otps = {}

            stctx = {}

            def st_setup(g):
                """Allocate group tiles and define the ST/exp/wacc emitters.
                GV trims the final group's padded query columns (cols beyond
                SEQ) out of every pass."""
                i0, G = GROUPS[g]
                GW = G * 128
                GV = min(GW, SEQ - i0 * 128)
                eg = egp.tile([128, NB, GW], F16, tag="eg", name=f"eg{g}")
                w_ps = wps.tile([32, GV], F32, tag="wps", name=f"wps{g}")
                egs[g] = eg
                stiles = {}

                def emit_st(t):
                    stile = stpool.tile([128, 1024], F32, tag="st",
                                        name=f"st{g}_{t}")
                    stiles[t] = stile
                    for n, j in enumerate(PAIRS[t]):
                        nc.tensor.matmul(
                            stile[:, 512 * n: 512 * n + GV],
                            sk[:, j * 128:(j + 1) * 128],
                            sq[:, i0 * 128:i0 * 128 + GV],
                            start=True, stop=True,
                        )

                def emit_exp(t):
                    stile = stiles.pop(t)
                    pair = PAIRS[t]
                    if len(pair) == 2:
                        if GV == 512:
                            src = stile[:, 0:1024]
                        else:
                            src = _ap(stile[:, 0:1], [[512, 2], [1, GV]])
                        dst = eg[:, pair[0]:pair[0] + 2, 0:GV]
                    else:
                        src = stile[:, 0:GV]
                        dst = eg[:, pair[0], 0:GV]
                    nc.scalar.activation(
                        dst, src, mybir.ActivationFunctionType.Exp,
                        bias=0.0, scale=float(SCALE),
                    )

                def emit_wacc(t):
                    for j in PAIRS[t]:
                        nc.tensor.matmul(
                            w_ps[:, :], sbas[:, j, :], eg[:, j, 0:GV],
                            start=(j == 0), stop=(j == NB - 1),
                        )

                stctx[g] = (emit_st, emit_exp, emit_wacc, w_ps, GV)

            def st_prefix(g):
                """First two ST pairs + first exp, emitted inside the previous
                group's chain so the exp pipeline never drains."""
                st_setup(g)
                emit_st, emit_exp, _, _, _ = stctx[g]
                emit_st(0)
                emit_st(1)
                emit_exp(0)

            def st_main(g, pvq):
                """Remaining pairs, pipelined; drains pv thunks into slack."""
                emit_st, emit_exp, emit_wacc, w_ps, GV = stctx[g]
                np_ = len(PAIRS)
                for t in range(2, np_):
                    emit_st(t)
                    emit_exp(t - 1)
                    if t <= 12:
                        for _ in range(2):
                            if pvq:
                                pvq.pop(0)()
                    emit_wacc(t - 2)
                emit_exp(np_ - 1)
                emit_wacc(np_ - 2)
                emit_wacc(np_ - 1)

                w_sb = gwp.tile([32, GV], F32, tag="wsb", name=f"wsb{g}")
                nc.vector.tensor_copy(w_sb[:, :], w_ps[:, :])
                wsbs[g] = w_sb

            def take(pvq, n):
                for _ in range(n):
                    if pvq:
                        pvq.pop(0)()

            def chain(g, pvq, light=False):
                """Energy-mask chain for group g (tiny PE transposes + DVE
                compare ops), with leftover pv thunks filling PE slack. With
                `light`, keep most thunks for after the mask-row broadcast so
                the PE stays covered while the final mask TTs run on DVE."""
                i0, G = GROUPS[g]
                w_sb = wsbs[g]
                nt = 1 if light else 3
                take(pvq, nt)

                # W^T (+ den in column 31 of each 32-block) per query block
                wt4_ps = minips.tile([128, 128], F32, tag="mini")
                for il in range(G):
                    i = i0 + il
                    nv = NVLAST if i == 30 else 128
                    qs0 = il * 128
                    nc.tensor.transpose(wt4_ps[:nv, 32 * il:32 * il + 32],
                                        w_sb[:, qs0:qs0 + nv], i32[:, :])
                wt4 = small.tile([128, 128], F32, tag="wt4", name=f"wt4_{g}")
                R4 = small.tile([128, 128], F32, tag="R4", name=f"R4_{g}")
                rdw4 = small.tile([128, 4], F32, tag="rdw4", name=f"rdw4_{g}")
                nc.vector.memset(R4[:, :], 0.0)
                for il in range(G):
                    nv = NVLAST if i0 + il == 30 else 128
                    nc.vector.tensor_copy(wt4[:nv, 32 * il:32 * il + 32],
                                          wt4_ps[:nv, 32 * il:32 * il + 32])
                for il in range(G):
                    nv = NVLAST if i0 + il == 30 else 128
                    nc.vector.reciprocal(rdw4[:nv, il:il + 1],
                                         wt4[:nv, 32 * il + 31:32 * il + 32])
                for il in range(G):
                    nv = NVLAST if i0 + il == 30 else 128
                    nc.vector.scalar_tensor_tensor(
                        R4[:nv, 32 * il:32 * il + 31],
                        wt4[:nv, 32 * il:32 * il + 31],
                        rdw4[:nv, il:il + 1], onesb31[:nv, :],
                        mybir.AluOpType.mult, mybir.AluOpType.mult)
                take(pvq, nt)

                # pooling columns (stacked at 32-offsets) and rows
                pcol_ps = minips.tile([128, 1], F32, tag="mini")
                nc.tensor.matmul(pcol_ps[:, :], R4[:, :], ones_col128[:, :],
                                 start=True, stop=True)
                prow_ps = minips.tile([1, 128], F32, tag="mini")
                nc.tensor.matmul(prow_ps[:, :], ones_col128[:, :], R4[:, :],
                                 start=True, stop=True)
                pcol4 = small.tile([128, 1], F32, tag="pcol4", name=f"pc{g}")
                nc.vector.tensor_copy(pcol4[:, :], pcol_ps[:, :])
                prow = small.tile([1, 128], F32, tag="prow", name=f"pr{g}")
                nc.vector.tensor_copy(prow[:, :], prow_ps[:, :])
                take(pvq, nt)

                # Pb: rows 32*il..32*il+30 = pooling row of query block i0+il
                pb_ps = minips.tile([128, 31], F32, tag="mini")
                nc.vector.memset(pb_ps[:, :], 0.0)
                for il in range(G):
                    nc.tensor.matmul(pb_ps[32 * il:32 * il + 31, :],
                                     ones_row31[:, :],
                                     prow[:, 32 * il:32 * il + 31],
                                     start=True, stop=True,
                                     tile_position=(0, 32 * il))
                pb = small.tile([128, 31], F32, tag="pb", name=f"pb{g}")
                nc.vector.tensor_copy(pb[:, :], pb_ps[:, :])
                Gt = small.tile([128, 31], F32, tag="Gt", name=f"Gt{g}")
                rank = small.tile([128, 1], F32, tag="rank", name=f"rk{g}")
                nc.vector.scalar_tensor_tensor(
                    Gt[:, :], pb[:, :], pcol4[:, :], onesb31[:, :],
                    mybir.AluOpType.is_gt, mybir.AluOpType.mult,
                    accum_out=rank[:, :])
                esum = small.tile([128, 1], F32, tag="esum", name=f"es{g}")
                tmp = small.tile([128, 31], F32, tag="tmp", name=f"tp{g}")
                nc.vector.scalar_tensor_tensor(
                    tmp[:, :], pb[:, :], pcol4[:, :], pb[:, :],
                    mybir.AluOpType.is_ge, mybir.AluOpType.mult,
                    accum_out=esum[:, :])
                tot = small.tile([128, 1], F32, tag="tot", name=f"tt{g}")
                nc.vector.reduce_sum(tot[:, :], pb[:, :],
                                     axis=mybir.AxisListType.X)
                C = small.tile([128, 1], F32, tag="C", name=f"C{g}")
                nc.vector.scalar_tensor_tensor(
                    C[:, :], tot[:, :], 0.95, esum[:, :],
                    mybir.AluOpType.mult, mybir.AluOpType.is_gt)
                ca = small.tile([128, 1], F32, tag="ca", name=f"ca{g}")
                nc.vector.scalar_tensor_tensor(
                    ca[:, :], rank[:, :], 21.0, C[:, :],
                    mybir.AluOpType.is_lt, mybir.AluOpType.logical_and)
                mv4 = small.tile([128, 1], F32, tag="mv4", name=f"mv{g}")
                nc.vector.scalar_tensor_tensor(
                    mv4[:, :], rank[:, :], 1.0, ca[:, :],
                    mybir.AluOpType.is_lt, mybir.AluOpType.logical_or)
                take(pvq, nt)

                # mask rows -> partition-broadcast tile. The broadcast is a
                # K=128 fp16 matmul (column sums of [mask row; zeros]) --
                # single-pass, unlike a K=1 fp32 outer product.
                mrow_ps = minips.tile([1, 128], F32, tag="mini")
                nc.tensor.transpose(mrow_ps[:, :], mv4[:, :], i128[:, :])
                nc.vector.tensor_copy(mrowB[0:1, :], mrow_ps[:, :])
                mb_ps = minips.tile([128, 127], F32, tag="mini")
                nc.tensor.matmul(mb_ps[:, :], onesq128[:, :],
                                 mrowB[0:128, 0:127], start=True, stop=True)
                take(pvq, len(pvq))
                mbc4 = small.tile([128, 127], F16, tag="mbc4", name=f"mb{g}")
                nc.vector.tensor_copy(mbc4[:, :], mb_ps[:, :])
                # pair-duplicated mask: mdup[p, 64*il + 2j + e] = mask_il[j]
                mdup = small.tile([128, 256], F16, tag="mdup", name=f"md{g}")
                for e in range(2):
                    nc.vector.tensor_copy(
                        _ap(mdup[:, e:e + 1], [[64, G], [2, 31]]),
                        _ap(mbc4[:, 0:1], [[32, G], [1, 31]]))
                mbcs[g] = (mbc4, mdup, wt4)

            def emit_mask(g):
                """In-place E *= mask; pair-packed APs (all last dims unit
                stride, 2-byte) enable the DVE 2x performance mode."""
                i0, G = GROUPS[g]
                GW = G * 128
                GV = min(GW, SEQ - i0 * 128)
                eg = egs[g]
                mdup = mbcs[g][1]
                for il in range(G):
                    qs0 = il * 128
                    qc = min(128, GV - qs0)
                    ev = _ap(eg[:, 0, qs0:qs0 + 1],
                             [[GW, NB], [2, qc // 2], [1, 2]])
                    mv = _ap(mdup[:, 64 * il:64 * il + 1],
                             [[2, NB], [0, qc // 2], [1, 2]])
                    nc.vector.tensor_tensor(
                        ev, ev, mv, mybir.AluOpType.mult)

            def emit_den(g):
                """Masked denominator column per query block (after the mask
                TTs so those start as early as possible on DVE)."""
                i0, G = GROUPS[g]
                mbc4, _, wt4 = mbcs[g]
                for il in range(G):
                    i = i0 + il
                    nv = NVLAST if i == 30 else 128
                    dtmp = small.tile([128, 31], F32, tag="dtmp",
                                      name=f"dt{g}_{il}")
                    nc.vector.scalar_tensor_tensor(
                        dtmp[:nv, :], wt4[:nv, 32 * il:32 * il + 31], 1.0,
                        mbc4[:nv, 32 * il:32 * il + 31],
                        mybir.AluOpType.mult, mybir.AluOpType.mult,
                        accum_out=den_sb[:nv, i:i + 1])
                nc.sync.dma_start(den_d[:, i0:i0 + G], den_sb[:, i0:i0 + G])

            def make_pv_thunks(g, per_il=False):
                """PV matmuls for group g as thunks; creates the PSUM tile."""
                i0, G = GROUPS[g]
                GV = min(G * 128, SEQ - i0 * 128)
                eg = egs[g]
                ot_ps = pvps.tile([128, GV], F32, tag="ot", name=f"ot{g}")
                otps[g] = ot_ps
                thunks = []
                if per_il:
                    for il in range(G):
                        qs0 = il * 128
                        for j in range(NB):
                            def th(j=j, qs0=qs0):
                                nc.tensor.matmul(
                                    ot_ps[:, qs0:qs0 + 128],
                                    sv[:, j, :], eg[:, j, qs0:qs0 + 128],
                                    start=(j == 0), stop=(j == NB - 1))
                            thunks.append(th)
                else:
                    for j in range(NB):
                        def th(j=j):
                            nc.tensor.matmul(
                                ot_ps[:, :], sv[:, j, :], eg[:, j, 0:GV],
                                start=(j == 0), stop=(j == NB - 1))
                        thunks.append(th)
                return thunks

            def emit_out(g):
                i0, G = GROUPS[g]
                GV = min(G * 128, SEQ - i0 * 128)
                ot_sb = outsp.tile([128, GV], F16, tag="otsb", name=f"ob{g}")
                nc.vector.tensor_copy(ot_sb[:, :], otps[g][:, :])
                nc.sync.dma_start(outT_d[:, i0 * 128:i0 * 128 + GV],
                                  ot_sb[:, :])
                del egs[g], otps[g], mbcs[g]

            # ---- software-pipelined schedule (PV lags ST by two groups so
            # the mask multiply has a full iteration of DVE slack) ----
            ng = len(GROUPS)
            prev1 = []   # PV thunks of group g-1
            prev2 = []   # PV thunks of group g-2, consumed this iter
            for g in range(ng):
                st_prefix(g)
                st_main(g, prev2)
                if g == ng - 1:
                    # Last chain: also drain PV(g-1) so its matmuls queue
                    # after the mask-row broadcast, covering the PE while the
                    # final mask multiplies run on DVE.
                    prev2.extend(prev1)
                    prev1 = []
                chain(g, prev2, light=(g == ng - 1))
                if g >= 2:
                    emit_out(g - 2)
                emit_mask(g)
                emit_den(g)
                if g < ng - 1:
                    prev2 = prev1
                    prev1 = make_pv_thunks(g)
                else:
                    last = prev2
                    pv_last = make_pv_thunks(g)
            for th in last:
                th()
            emit_out(ng - 2)
            for th in pv_last:
                th()
            emit_out(ng - 1)

    nc.compile()
    return nc


_CACHED = {}


def _get_graph():
    if "nc" not in _CACHED:
        _CACHED["nc"] = build_graph()
    return _CACHED["nc"]


def _prepare_inputs(q, k, v, perm):
    q = np.asarray(q, dtype=np.float32)
    k = np.asarray(k, dtype=np.float32)
    v = np.asarray(v, dtype=np.float32)
    perm = np.asarray(perm, dtype=np.int64)

    def rearr(x):  # [1,8,SEQ,D] -> video permuted first, text appended
        return np.concatenate([x[0, :, TEXT:, :][:, perm, :], x[0, :, :TEXT, :]],
                              axis=1)

    qr, kr, vr = rearr(q), rearr(k), rearr(v)      # [8, SEQ, D]
    i32 = np.eye(32, dtype=np.float32)
    i128 = np.eye(128, dtype=np.float32)
    # basis: bas[r, j, m] = 1 if (m == j or m == 31) and key row r of block j
    # is valid; column 31 accumulates the full softmax denominator.
    bas = np.zeros((128, NB, 32), np.float16)
    for j in range(NB):
        kv = NVLAST if j == 30 else 128
        bas[:kv, j, j] = 1.0
        bas[:kv, j, 31] = 1.0
    bas = np.ascontiguousarray(bas.reshape(128, NB * 32))
    in_maps = []
    for c in range(NCORES):
        qp = np.zeros((SP, D), np.float16)
        qp[:SEQ] = qr[c]
        kp = np.zeros((SP, D), np.float16)
        kp[:SEQ] = kr[c]
        vp = np.zeros((SP, D), np.float16)
        vp[:SEQ] = vr[c]
        in_maps.append({
            "qT": np.ascontiguousarray(qp.T),
            "kT": np.ascontiguousarray(kp.T),
            "vv": np.ascontiguousarray(
                vp.reshape(NB, 128, D).transpose(1, 0, 2).reshape(128, NB * 128)),
            "bas": bas,
            "i32": i32,
            "i128": i128,
        })
    return in_maps, perm


def run(inputs, trace=False, trace_kwargs=None):
    nc = _get_graph()
    in_maps, perm = _prepare_inputs(inputs["q"], inputs["k"], inputs["v"],
                                    inputs["perm"])
    res = run_bass_kernel_spmd(
        nc, in_maps, core_ids=list(range(NCORES)), trace=trace,
        **(trace_kwargs or {}),
    )
    outs = np.empty((NCORES, SEQ, D), np.float32)
    for c in range(NCORES):
        oT = res.results[c]["outT"][:, :SEQ].astype(np.float32)  # [D, SEQ]
        den = res.results[c]["den"].T.reshape(SP)[:SEQ]   # den[q%128, i] -> [SEQ]
        outs[c] = (oT / den[None, :]).T
    g2o = np.argsort(perm)
    txt = outs[:, VID:SEQ, :]
    vid = outs[:, :VID, :][:, g2o, :]
    full = np.concatenate([txt, vid], axis=1)[None]   # [1, 8, SEQ, D]
    return np.ascontiguousarray(full.astype(np.float32)), res


def kernel(q, k, v, perm):
    out, _ = run({"q": q, "k": k, "v": v, "perm": perm})
    return out
